# revision 23
# baseline (speedup 1.0000x reference)
"""Trainium2 Bass kernel for nn_AttnBlock (VAE-style spatial attention block).

Reference computation (per batch b):
  h = LayerNorm_C(x) * ln_w + ln_b            (channels-first LN over C)
  q = conv1x1(h, wq); k = conv3x3(h, wk); v = conv3x3(h, wv)   (pad 1)
  attn = softmax_n(q^T k / sqrt(C));  out = v @ attn^T
  y = x + conv1x1(out, wp) + bp

Sharding: 8 cores; core i -> batch i//2, KEY half i%2.  Each core:
  * LN over its 34-row xkv strip,
  * composite convs on its 2048 key pixels:
      k2 = (Wq . Wk) (*) h      (the 1x1 q-conv folded into the k conv)
      v2 = (Wp . Wv) (*) h      (the 1x1 proj folded into the v conv)
  * LN for the non-owned queries (owned queries reuse the strip tile),
  * exp-scores against its keys, the unnormalized numerator Z = v2 @ p^T,
    and the partial denominator l = sum(p).
The host merges each pair exactly: y = x + (Z_a + Z_b)/(l_a + l_b) + bp.

v3: all heavy matmuls run in fp8e4 (e4m3) with DoubleRow perf mode
(256-deep contraction at 0.5 cycles/row).  The 3x3 convs are direct
(9 taps x 2 ct-pairs accumulated in PSUM) with 4D moving-window APs over
the zero-padded strip.  l comes from a DoubleRow ones-row matmul.
Inputs arrive bf16, z leaves bf16.  Weight/feature scaling keeps every
fp8 tensor in e4m3's normal range (see SCALES below).
"""

import os

os.environ.setdefault("MYCRO_LOCAL_CACHE", "1")

import numpy as np
import ml_dtypes

import concourse.bacc as bacc
import concourse.mybir as mybir
import concourse.tile as tile

F32 = mybir.dt.float32
F32R = mybir.dt.float32r
BF16 = mybir.dt.bfloat16
F8 = mybir.dt.float8e4
AF = mybir.ActivationFunctionType
OP = mybir.AluOpType
DR = mybir.MatmulPerfMode.DoubleRow
EPS = 1e-6

# fp8 scale plan (folded on host / into copy scales):
#   wk2 host-scaled by 2^9  -> k_sb holds 2^9 * k2    (std ~22)
#   exp applies scale 2^-9 on the score PSUM
#   wv2 host-scaled by 2^5  -> vT_sb holds 2^5 * v2   (std ~32)
#   z copy applies 2^-5; exp bias -2 cancels in the host's Z/l division
WKS = 2.0**9
WVS = 2.0**5
EXPB = -2.0


def build_attn_kernel(C=512, H=64, W=64, lnb_zero=False):
    HW = H * W
    KH = H // 2                  # key rows owned by this core
    KVR = KH + 2                 # strip rows incl. 2 context rows
    KHW = KH * W                 # key pixels owned
    CT = C // 128                # channel tiles
    NT = KHW // 128              # key-pixel tiles (this core)
    PW = W + 2                   # zero-padded row width
    MC = 512                     # query-chunk size
    NCH = HW // MC               # query chunks (all pixels)
    RS = MC // W                 # rows per chunk / conv slab (8)
    NSLAB = KH // RS             # conv slabs (4)
    HQ = HW // 2                 # non-owned query pixels
    assert KHW % 128 == 0 and HW % MC == 0 and KH % RS == 0

    nc = bacc.Bacc("TRN2")

    xq_d = nc.dram_tensor("xq", (C, HQ), BF16, kind="ExternalInput")
    xkv_d = nc.dram_tensor("xkv", (C, KVR * W), BF16, kind="ExternalInput")
    wk_d = nc.dram_tensor("wk", (128, 9 * CT, C), F8, kind="ExternalInput")
    wv_d = nc.dram_tensor("wv", (128, 9 * CT, C), F8, kind="ExternalInput")
    lnb_d = nc.dram_tensor("lnb", (C, 1), F32, kind="ExternalInput")
    z_d = nc.dram_tensor("z", (C, HW), BF16, kind="ExternalOutput")
    l_d = nc.dram_tensor("l", (1, HW), F32, kind="ExternalOutput")

    with tile.TileContext(nc) as tc:
        with (
            tc.tile_pool(name="consts", bufs=1) as consts,
            tc.tile_pool(name="persist", bufs=1) as persist,
        ):
            # persistent SBUF state
            hkv_sb = persist.tile((128, CT, KVR, PW), F8)        # padded LN'd strip
            k_sb = persist.tile((128, CT, KHW), F8)              # 2^9 * k2  [c, pix]
            k_lo = persist.tile((128, CT, KHW), F8)              # fp8 residual of k_sb
            vT_sb = persist.tile((128, NT, C), F8)               # 2^5 * v2T [pix, c]
            vT_lo = persist.tile((128, NT, C), F8)               # fp8 residual of vT_sb
            qh_sb = persist.tile((128, CT, HQ), F8)              # non-owned queries
            nc.gpsimd.memset(hkv_sb, 0.0)

            onesf = consts.tile((128, 8), F32)
            nc.vector.memset(onesf, 1.0 / C)
            ones_bf = consts.tile((128, 1), BF16)                # value 1/C
            nc.vector.tensor_copy(ones_bf, onesf[:, 0:1])
            ones8 = consts.tile((128, 2, 128), F8)               # DoubleRow ones block
            nc.vector.memset(ones8, 1.0)
            eps_t = consts.tile((1, 1), F32)
            nc.vector.memset(eps_t, EPS)
            expb_t = consts.tile((128, 1), F32)
            nc.vector.memset(expb_t, EXPB)
            lnb_sb = consts.tile((128, CT), F32)
            from concourse.masks import make_identity
            ident_f = consts.tile((128, 128), F32)
            make_identity(nc, ident_f)
            ident = consts.tile((128, 128), BF16)
            nc.vector.tensor_copy(ident, ident_f)

            # ---- LN helper: one chunk of pixels; mean/E[x^2] via bf16 ones
            # matmuls on PE, apply on DVE (+Pool for half the subs).
            def ln_chunk(src_dram, sl, KC, P, out_ap_fn, out_rearrange=None,
                         nsplit=2):
                io, tmp, ps, bcp = P
                xs = io.tile((128, CT, MC), BF16, tag="xs", name="xs")[:, :, :KC]
                xs_src = src_dram[:, sl].rearrange("(t p) n -> p t n", p=128)
                step = CT // nsplit
                for j in range(0, CT, step):
                    nc.sync.dma_start(out=xs[:, j : j + step],
                                      in_=xs_src[:, j : j + step])
                xsq = tmp.tile((128, CT, MC), BF16, tag="xsq", name="xsq",
                               bufs=1)[:, :, :KC]
                # mean on partition 0, E[x^2] on partition 32: one PSUM bank
                mm = ps.tile((33, MC), F32, tag="mm", name="mm")
                mean = mm[0:1, :KC]
                msq = mm[32:33, :KC]
                for t in range(CT):
                    nc.tensor.matmul(mean, ones_bf, xs[:, t],
                                     start=(t == 0), stop=(t == CT - 1))
                for t in range(CT):
                    nc.vector.tensor_mul(xsq[:, t], xs[:, t], xs[:, t])
                    nc.tensor.matmul(msq, ones_bf, xsq[:, t],
                                     start=(t == 0), stop=(t == CT - 1))
                m2 = tmp.tile((1, MC), F32, tag="m2", name="m2", bufs=1)[:, :KC]
                nc.scalar.square(m2, mean)
                var = tmp.tile((1, MC), F32, tag="var", name="var", bufs=1)[:, :KC]
                nc.vector.tensor_sub(var, msq, m2)
                rstd = tmp.tile((1, MC), F32R, tag="rstd", name="rstd",
                                bufs=1)[:, :KC]
                nc.scalar.activation(rstd, var, AF.Sqrt, bias=eps_t)
                with nc.allow_low_precision(reason="f32r rstd broadcast"):
                    nc.vector.reciprocal(rstd, rstd)
                nmr = tmp.tile((1, MC), F32R, tag="nmr", name="nmr", bufs=1)[:, :KC]
                nc.vector.tensor_mul(nmr, mean, rstd)
                bc0 = bcp.tile((128, MC), F32R, tag="bc0", name="bc0")[:, :KC]
                nc.gpsimd.partition_broadcast(bc0, rstd, channels=128)
                bc1 = bcp.tile((128, MC), F32R, tag="bc1", name="bc1")[:, :KC]
                nc.gpsimd.partition_broadcast(bc1, nmr, channels=128)
                for t in range(CT):
                    hmul = tmp.tile((128, MC), F32R, tag="hmul", name="hmul",
                                    bufs=1)[:, :KC]
                    nc.vector.tensor_mul(hmul, xs[:, t], bc0)
                    out_ap = out_ap_fn(t)
                    if out_rearrange is not None:
                        pat, kw = out_rearrange
                        h_in = hmul.rearrange(pat, **kw)
                        b_in = bc1.rearrange(pat, **kw)
                    else:
                        h_in, b_in = hmul, bc1
                    if lnb_zero:
                        eng = nc.gpsimd if t >= 2 else nc.vector
                        eng.tensor_sub(out_ap, h_in, b_in)
                    else:
                        nc.vector.scalar_tensor_tensor(
                            out_ap, h_in, lnb_sb[:, t : t + 1], b_in,
                            op0=OP.add, op1=OP.subtract,
                        )

            # PSUM budget (8 banks): mean 1 + msq 1 (reused by l) +
            # m0 1 + m1 1 (conv, reused by po) + pvt 1 + ps 3 = 8.
            with (
                tc.tile_pool(name="xio", bufs=2) as xio,
                tc.tile_pool(name="ltmp", bufs=2) as ltmp,
                tc.tile_pool(name="lbc", bufs=1) as lbc,
                tc.tile_pool(name="cwp", bufs=1) as cwp,
                tc.tile_pool(name="vsl", bufs=2) as vsl,
                tc.tile_pool(name="app", bufs=40) as app,
                tc.tile_pool(name="zout", bufs=2) as zout,
                tc.tile_pool(name="lout", bufs=2) as lout,
                tc.tile_pool(name="lps", bufs=1, space="PSUM") as lps,
                tc.tile_pool(name="kps", bufs=1, space="PSUM") as kps,
                tc.tile_pool(name="aps", bufs=3, space="PSUM") as aps,
            ):
                P = [xio, ltmp, lps, lbc]

                # ---- strip LN chunks -> padded hkv tile
                strip_chunks = []
                done = 0
                while done < KVR * W:
                    KC = min(MC, KVR * W - done)
                    strip_chunks.append((done, KC))
                    done += KC

                def emit_strip_chunk(c, nsplit=2):
                    off, KC = c
                    r0, nr = off // W, KC // W
                    ln_chunk(
                        xkv_d[:], slice(off, off + KC), KC, P,
                        lambda t: hkv_sb[:, t, r0 : r0 + nr, 1 : W + 1],
                        out_rearrange=("p (r w) -> p r w", dict(w=W)),
                        nsplit=nsplit,
                    )

                # LN chunk for non-owned queries -> qh_sb
                def emit_q_chunk(lc):
                    lsl = slice(lc * MC, (lc + 1) * MC)
                    ln_chunk(xq_d[:], lsl, MC, P, lambda t: qh_sb[:, t, lsl])

                # ---- direct 3x3 conv, one slab = RS rows x W cols = MC pix.
                # moving = 4D padded window; 18 DoubleRow accumulation steps.
                def emit_conv_slab(w_sb, s, dest_fn):
                    for ot in range(CT):
                        pk = kps.tile((128, MC), F32, tag=f"m{ot % 2}",
                                      name="pk")
                        i = 0
                        for dy in range(3):
                            for dx in range(3):
                                win = hkv_sb[:, :, RS * s + dy : RS * s + dy + RS,
                                             dx : dx + W]
                                for tp in range(CT // 2):
                                    nc.tensor.matmul(
                                        pk,
                                        w_sb[:, (dy * 3 + dx) * CT + 2 * tp :
                                             (dy * 3 + dx) * CT + 2 * tp + 2,
                                             ot * 128 : ot * 128 + 128],
                                        win[:, 2 * tp : 2 * tp + 2],
                                        start=(i == 0), stop=(i == 17),
                                        perf_mode=DR,
                                    )
                                    i += 1
                        dest_fn(ot, pk)

                def k_dest(s):
                    ksl = slice(s * MC, (s + 1) * MC)
                    def dest(ot, pk):
                        nc.vector.tensor_copy(k_sb[:, ot, ksl], pk)
                        # fp8 residual: writing the sub result to fp8 IS the
                        # second-level quantization
                        nc.vector.tensor_sub(k_lo[:, ot, ksl], pk,
                                             k_sb[:, ot, ksl])
                    return dest

                def emit_v_slab(s):
                    vslab = vsl.tile((128, CT, MC), BF16, tag="vslab",
                                     name="vslab")

                    def dest(ot, pk):
                        nc.scalar.copy(vslab[:, ot], pk)

                    emit_conv_slab(wv_sb, s, dest)
                    for blk in range(MC // 128):
                        n_idx = s * (MC // 128) + blk
                        pvt = kps.tile((128, C), BF16, tag="pvt",
                                       name="pvt", bufs=1)
                        for ct in range(CT):
                            nc.tensor.transpose(
                                pvt[:, ct * 128 : ct * 128 + 128],
                                vslab[:, ct, blk * 128 : (blk + 1) * 128], ident)
                        nc.scalar.copy(vT_sb[:, n_idx], pvt)
                        nc.vector.tensor_sub(vT_lo[:, n_idx], pvt,
                                             vT_sb[:, n_idx])

                # ---- attention chunk pieces (LOCAL chunk indexing: chunks
                # 0..3 are this core's own key-half rows (read from the strip
                # tile), 4..7 the other half (read from qh_sb).  The host
                # permutes z/l back to global order per core half.
                def q_src(i, tp):
                    if i < NCH // 2:
                        r = RS * i + 1
                        return hkv_sb[:, 2 * tp : 2 * tp + 2, r : r + RS,
                                      1 : W + 1]
                    lc = i - NCH // 2
                    return qh_sb[:, 2 * tp : 2 * tp + 2,
                                 lc * MC : (lc + 1) * MC]

                p_stash = {}

                def emit_scores(i):
                    ps_l = []
                    for j in range(NT // 2):
                        p2 = app.tile((128, 2, MC), F8, tag="p", name="p2")
                        ps2 = aps.tile((128, 2, MC), F32, tag="ps", name="ps2",
                                       bufs=2)
                        for h2 in range(2):
                            n = 2 * j + h2
                            st = 0
                            for klv in (k_sb, k_lo):
                                for tp in range(CT // 2):
                                    nc.tensor.matmul(
                                        ps2[:, h2], klv[:, 2 * tp : 2 * tp + 2,
                                                        n * 128 : (n + 1) * 128],
                                        q_src(i, tp),
                                        start=(st == 0), stop=(st == CT - 1),
                                        perf_mode=DR,
                                    )
                                    st += 1
                        # one exp covers both halves of the pair
                        nc.scalar.activation(
                            p2.rearrange("p a n -> p (a n)"),
                            ps2.rearrange("p a n -> p (a n)"),
                            AF.Exp, bias=expb_t, scale=1.0 / WKS)
                        ps_l.append(p2)
                    p_stash[i] = ps_l

                def emit_pv(i):
                    msl = slice(i * MC, (i + 1) * MC)
                    ps_l = p_stash.pop(i)
                    for ct in range(CT):
                        po = kps.tile((128, MC), F32, tag=f"m{ct % 2}",
                                      name="po", bufs=1)
                        st = 0
                        for j in range(NT // 2):
                            for vlv in (vT_sb, vT_lo):
                                nc.tensor.matmul(
                                    po, vlv[:, 2 * j : 2 * j + 2,
                                            ct * 128 : ct * 128 + 128],
                                    ps_l[j], start=(st == 0), stop=(st == NT - 1),
                                    perf_mode=DR,
                                )
                                st += 1
                        z_sb = zout.tile((128, MC), BF16, tag=f"z{ct % 2}",
                                         name="z_sb")
                        nc.vector.tensor_copy(z_sb, po)  # 2^-5 folded on host
                        nc.sync.dma_start(
                            out=z_d[ct * 128 : ct * 128 + 128, msl], in_=z_sb)
                    # every output row of this DoubleRow matmul is sum_n p;
                    # reuses the m0 conv/PV psum bank, row 0 is copied out.
                    l_ps = kps.tile((128, MC), F32, tag="m0", name="l_ps",
                                    bufs=1)
                    for j in range(NT // 2):
                        nc.tensor.matmul(l_ps, ones8, ps_l[j],
                                         start=(j == 0), stop=(j == NT // 2 - 1),
                                         perf_mode=DR)
                    l_sb = lout.tile((1, MC), F32, tag="lsb", name="l_sb")
                    nc.scalar.copy(l_sb, l_ps[0:1])
                    nc.sync.dma_start(out=l_d[:, msl], in_=l_sb)

                # ---- emission schedule ------------------------------------
                nc.sync.dma_start(
                    out=lnb_sb, in_=lnb_d[:].rearrange("(t p) o -> p (t o)", p=128)
                )
                wk_sb = cwp.tile((128, 9 * CT, C), F8, tag="cw", name="wk_sb")
                emit_strip_chunk(strip_chunks[0], nsplit=4)
                nc.sync.dma_start(out=wk_sb[:, : 3 * CT], in_=wk_d[:, : 3 * CT])
                emit_strip_chunk(strip_chunks[1], nsplit=4)
                nc.sync.dma_start(out=wk_sb[:, 3 * CT : 6 * CT],
                                  in_=wk_d[:, 3 * CT : 6 * CT])
                emit_strip_chunk(strip_chunks[2])
                nc.sync.dma_start(out=wk_sb[:, 6 * CT :], in_=wk_d[:, 6 * CT :])
                emit_strip_chunk(strip_chunks[3])
                emit_strip_chunk(strip_chunks[4])
                for lc in range(NSLAB):
                    emit_q_chunk(lc)

                for s in range(NSLAB):
                    emit_conv_slab(wk_sb, s, k_dest(s))

                wv_sb = cwp.tile((128, 9 * CT, C), F8, tag="cw2", name="wv_sb")
                for j in range(3):
                    nc.sync.dma_start(out=wv_sb[:, 3 * j * CT : 3 * (j + 1) * CT],
                                      in_=wv_d[:, 3 * j * CT : 3 * (j + 1) * CT])

                # scores for owned chunks overlap the v conv; PV waits on vT.
                for s in range(NSLAB):
                    emit_scores(s)
                    emit_v_slab(s)
                emit_pv(0)
                for s in range(NSLAB):
                    emit_scores(NCH // 2 + s)
                    emit_pv(s + 1)
                emit_pv(NCH // 2 + 1)
                emit_pv(NCH // 2 + 2)
                emit_pv(NCH // 2 + 3)

    nc.compile()
    return nc


_NC_CACHE = {}


def _get_nc(C, H, W, lnb_zero=False):
    key = (C, H, W, lnb_zero)
    if key not in _NC_CACHE:
        _NC_CACHE[key] = build_attn_kernel(C, H, W, lnb_zero=lnb_zero)
    return _NC_CACHE[key]


def make_in_maps(x, ln_w, ln_b, wq, wk, wv, wp, bp, n_cores=8):
    """Host-side prep: shard + relayout inputs for each core."""
    x = np.asarray(x, np.float32)
    B, C, H, W_ = x.shape
    HW = H * W_
    KH = H // 2
    CT = C // 128
    scale = float(C) ** -0.5
    lnw = np.asarray(ln_w, np.float32).reshape(C)
    F8NP = ml_dtypes.float8_e4m3

    # composite conv weights: k2 = (wq*scale . wk) (*) h, v2 = (wp . wv) (*) h
    wq2 = np.asarray(wq, np.float32)[:, :, 0, 0] * scale       # [O, C]
    wpm = np.asarray(wp, np.float32)[:, :, 0, 0]               # [O, C]
    wk9 = np.asarray(wk, np.float32).reshape(C, C, 9)          # [O, I, tap]
    wv9 = np.asarray(wv, np.float32).reshape(C, C, 9)

    def _composite(m1, w9, s, transpose_m1):
        # w2[o, i, tap] = sum_c m1[c|o, o|c] * w9[c, i, tap] * lnw[i] * s
        ein = "co,cit->oit" if transpose_m1 else "oc,cit->oit"
        w2 = np.einsum(ein, m1, w9) * lnw[None, :, None] * s
        # layout [p, tap*CT + t, o] with i = t*128 + p
        arr = w2.transpose(1, 2, 0).reshape(CT, 128, 9, C)     # [t, p, tap, o]
        arr = arr.transpose(1, 2, 0, 3).reshape(128, 9 * CT, C)
        return np.ascontiguousarray(arr.astype(F8NP))

    # logits = h_q^T (Wq^T k), so the q fold uses Wq TRANSPOSED; the proj
    # fold (out = Wp attn_out) uses Wp as-is.
    wkT = _composite(wq2, wk9, WKS, transpose_m1=True)
    wvT = _composite(wpm, wv9, WVS, transpose_m1=False)
    lnb = np.ascontiguousarray(np.asarray(ln_b, np.float32).reshape(C, 1))
    xi = x.reshape(B, C, H, W_)
    in_maps = []
    for core in range(n_cores):
        b, half = divmod(core, 2)
        b = b % B
        zero = np.zeros((C, 1, W_), np.float32)
        if half == 0:
            strip = np.concatenate([zero, xi[b][:, 0 : KH + 1]], axis=1)
            xq = xi[b][:, KH:H]
        else:
            strip = np.concatenate([xi[b][:, KH - 1 : H], zero], axis=1)
            xq = xi[b][:, 0:KH]
        in_maps.append({
            "xq": np.ascontiguousarray(
                xq.reshape(C, HW // 2).astype(ml_dtypes.bfloat16)),
            "xkv": np.ascontiguousarray(
                strip.reshape(C, (KH + 2) * W_).astype(ml_dtypes.bfloat16)),
            "wk": wkT, "wv": wvT, "lnb": lnb,
        })
    return in_maps


def merge_outputs(x, bp, results):
    """Exact pair-merge: y = x + (Z_a + Z_b) / (l_a + l_b) + bp.

    Cores write queries in LOCAL order (own key-half rows first); half-1
    cores therefore need their z/l swapped back to global row order."""
    x = np.asarray(x, np.float32)
    B, C, H, W_ = x.shape
    HW = H * W_
    hh = HW // 2
    bp = np.asarray(bp, np.float32).reshape(C, 1)

    def _glob(res, half):
        z = res["z"].astype(np.float32)
        l = np.asarray(res["l"], np.float32)
        if half == 1:
            z = np.concatenate([z[:, hh:], z[:, :hh]], axis=1)
            l = np.concatenate([l[:, hh:], l[:, :hh]], axis=1)
        return z, l

    out = np.empty((B, C, HW), np.float32)
    for b in range(B):
        za, la = _glob(results[2 * b], 0)
        zb, lb = _glob(results[2 * b + 1], 1)
        out[b] = x.reshape(B, C, HW)[b] + (za + zb) / (WVS * (la + lb)) + bp
    return out.reshape(B, C, H, W_)


def kernel(x, ln_w, ln_b, wq, wk, wv, wp, bp):
    from concourse.bass_utils import run_bass_kernel_spmd

    x = np.asarray(x, np.float32)
    B, C, H, W_ = x.shape
    lnb_zero = bool((np.asarray(ln_b, np.float32) == 0).all())
    nc = _get_nc(C, H, W_, lnb_zero=lnb_zero)
    in_maps = make_in_maps(x, ln_w, ln_b, wq, wk, wv, wp, bp)
    res = run_bass_kernel_spmd(nc, in_maps, core_ids=list(range(8)))
    return merge_outputs(x, bp, res.results)


# revision 24
# speedup vs baseline: 1.0261x; 1.0261x over previous
"""Trainium2 Bass kernel for nn_AttnBlock (VAE-style spatial attention block).

Reference computation (per batch b):
  h = LayerNorm_C(x) * ln_w + ln_b            (channels-first LN over C)
  q = conv1x1(h, wq); k = conv3x3(h, wk); v = conv3x3(h, wv)   (pad 1)
  attn = softmax_n(q^T k / sqrt(C));  out = v @ attn^T
  y = x + conv1x1(out, wp) + bp

Sharding: 8 cores; core i -> batch i//2, KEY half i%2.  Each core:
  * LN over its 34-row xkv strip,
  * composite convs on its 2048 key pixels:
      k2 = (Wq . Wk) (*) h      (the 1x1 q-conv folded into the k conv)
      v2 = (Wp . Wv) (*) h      (the 1x1 proj folded into the v conv)
  * LN for the non-owned queries (owned queries reuse the strip tile),
  * exp-scores against its keys, the unnormalized numerator Z = v2 @ p^T,
    and the partial denominator l = sum(p).
The host merges each pair exactly: y = x + (Z_a + Z_b)/(l_a + l_b) + bp.

v3: all heavy matmuls run in fp8e4 (e4m3) with DoubleRow perf mode
(256-deep contraction at 0.5 cycles/row).  The 3x3 convs are direct
(9 taps x 2 ct-pairs accumulated in PSUM) with 4D moving-window APs over
the zero-padded strip.  l comes from a DoubleRow ones-row matmul.
Inputs arrive bf16, z leaves bf16.  Weight/feature scaling keeps every
fp8 tensor in e4m3's normal range (see SCALES below).
"""

import os

os.environ.setdefault("MYCRO_LOCAL_CACHE", "1")

import numpy as np
import ml_dtypes

import concourse.bacc as bacc
import concourse.mybir as mybir
import concourse.tile as tile

F32 = mybir.dt.float32
F32R = mybir.dt.float32r
BF16 = mybir.dt.bfloat16
F8 = mybir.dt.float8e4
AF = mybir.ActivationFunctionType
OP = mybir.AluOpType
DR = mybir.MatmulPerfMode.DoubleRow
EPS = 1e-6

# fp8 scale plan (folded on host / into copy scales):
#   wk2 host-scaled by 2^9  -> k_sb holds 2^9 * k2    (std ~22)
#   exp applies scale 2^-9 on the score PSUM
#   wv2 host-scaled by 2^5  -> vT_sb holds 2^5 * v2   (std ~32)
#   z copy applies 2^-5; exp bias -2 cancels in the host's Z/l division
WKS = 2.0**9
WVS = 2.0**5
EXPB = -2.0


def build_attn_kernel(C=512, H=64, W=64, lnb_zero=False):
    HW = H * W
    KH = H // 2                  # key rows owned by this core
    KVR = KH + 2                 # strip rows incl. 2 context rows
    KHW = KH * W                 # key pixels owned
    CT = C // 128                # channel tiles
    NT = KHW // 128              # key-pixel tiles (this core)
    PW = W + 2                   # zero-padded row width
    MC = 512                     # query-chunk size
    NCH = HW // MC               # query chunks (all pixels)
    RS = MC // W                 # rows per chunk / conv slab (8)
    NSLAB = KH // RS             # conv slabs (4)
    HQ = HW // 2                 # non-owned query pixels
    assert KHW % 128 == 0 and HW % MC == 0 and KH % RS == 0

    nc = bacc.Bacc("TRN2")

    xq_d = nc.dram_tensor("xq", (C, HQ), BF16, kind="ExternalInput")
    xkv_d = nc.dram_tensor("xkv", (C, KVR * W), BF16, kind="ExternalInput")
    wk_d = nc.dram_tensor("wk", (128, 9 * CT, C), F8, kind="ExternalInput")
    wv_d = nc.dram_tensor("wv", (128, 9 * CT, C), F8, kind="ExternalInput")
    lnb_d = nc.dram_tensor("lnb", (C, 1), F32, kind="ExternalInput")
    z_d = nc.dram_tensor("z", (C, HW), BF16, kind="ExternalOutput")
    l_d = nc.dram_tensor("l", (1, HW), F32, kind="ExternalOutput")

    with tile.TileContext(nc) as tc:
        with (
            tc.tile_pool(name="consts", bufs=1) as consts,
            tc.tile_pool(name="persist", bufs=1) as persist,
        ):
            # persistent SBUF state
            hkv_sb = persist.tile((128, CT, KVR, PW), F8)        # padded LN'd strip
            k_sb = persist.tile((128, CT, KHW), F8)              # 2^9 * k2  [c, pix]
            k_lo = persist.tile((128, CT, KHW), F8)              # fp8 residual of k_sb
            vT_sb = persist.tile((128, NT, C), F8)               # 2^5 * v2T [pix, c]
            vT_lo = persist.tile((128, NT, C), F8)               # fp8 residual of vT_sb
            qh_sb = persist.tile((128, CT, HQ), F8)              # non-owned queries
            nc.gpsimd.memset(hkv_sb, 0.0)

            onesf = consts.tile((128, 8), F32)
            nc.vector.memset(onesf, 1.0 / C)
            ones_bf = consts.tile((128, 1), BF16)                # value 1/C
            nc.vector.tensor_copy(ones_bf, onesf[:, 0:1])
            ones8 = consts.tile((128, 2, 128), F8)               # DoubleRow ones block
            nc.vector.memset(ones8, 1.0)
            eps_t = consts.tile((1, 1), F32)
            nc.vector.memset(eps_t, EPS)
            expb_t = consts.tile((128, 1), F32)
            nc.vector.memset(expb_t, EXPB)
            lnb_sb = consts.tile((128, CT), F32)
            from concourse.masks import make_identity
            ident_f = consts.tile((128, 128), F32)
            make_identity(nc, ident_f)
            ident = consts.tile((128, 128), BF16)
            nc.vector.tensor_copy(ident, ident_f)

            # ---- LN helper: one chunk of pixels; mean/E[x^2] via bf16 ones
            # matmuls on PE, apply on DVE (+Pool for half the subs).
            def ln_chunk(src_dram, sl, KC, P, out_ap_fn, out_rearrange=None,
                         nsplit=2):
                io, tmp, ps, bcp = P
                xs = io.tile((128, CT, MC), BF16, tag="xs", name="xs")[:, :, :KC]
                xs_src = src_dram[:, sl].rearrange("(t p) n -> p t n", p=128)
                step = CT // nsplit
                for j in range(0, CT, step):
                    nc.sync.dma_start(out=xs[:, j : j + step],
                                      in_=xs_src[:, j : j + step])
                xsq = tmp.tile((128, CT, MC), BF16, tag="xsq", name="xsq",
                               bufs=1)[:, :, :KC]
                # mean on partition 0, E[x^2] on partition 32: one PSUM bank
                mm = ps.tile((33, MC), F32, tag="mm", name="mm")
                mean = mm[0:1, :KC]
                msq = mm[32:33, :KC]
                for t in range(CT):
                    nc.tensor.matmul(mean, ones_bf, xs[:, t],
                                     start=(t == 0), stop=(t == CT - 1))
                for t in range(CT):
                    nc.vector.tensor_mul(xsq[:, t], xs[:, t], xs[:, t])
                    nc.tensor.matmul(msq, ones_bf, xsq[:, t],
                                     start=(t == 0), stop=(t == CT - 1))
                m2 = tmp.tile((1, MC), F32, tag="m2", name="m2", bufs=1)[:, :KC]
                nc.scalar.square(m2, mean)
                var = tmp.tile((1, MC), F32, tag="var", name="var", bufs=1)[:, :KC]
                nc.vector.tensor_sub(var, msq, m2)
                rstd = tmp.tile((1, MC), F32R, tag="rstd", name="rstd",
                                bufs=1)[:, :KC]
                nc.scalar.activation(rstd, var, AF.Sqrt, bias=eps_t)
                with nc.allow_low_precision(reason="f32r rstd broadcast"):
                    nc.vector.reciprocal(rstd, rstd)
                nmr = tmp.tile((1, MC), F32R, tag="nmr", name="nmr", bufs=1)[:, :KC]
                nc.vector.tensor_mul(nmr, mean, rstd)
                bc0 = bcp.tile((128, MC), F32R, tag="bc0", name="bc0")[:, :KC]
                nc.gpsimd.partition_broadcast(bc0, rstd, channels=128)
                bc1 = bcp.tile((128, MC), F32R, tag="bc1", name="bc1")[:, :KC]
                nc.gpsimd.partition_broadcast(bc1, nmr, channels=128)
                for t in range(CT):
                    hmul = tmp.tile((128, MC), F32R, tag="hmul", name="hmul",
                                    bufs=1)[:, :KC]
                    nc.vector.tensor_mul(hmul, xs[:, t], bc0)
                    out_ap = out_ap_fn(t)
                    if out_rearrange is not None:
                        pat, kw = out_rearrange
                        h_in = hmul.rearrange(pat, **kw)
                        b_in = bc1.rearrange(pat, **kw)
                    else:
                        h_in, b_in = hmul, bc1
                    if lnb_zero:
                        eng = nc.gpsimd if t >= 2 else nc.vector
                        eng.tensor_sub(out_ap, h_in, b_in)
                    else:
                        nc.vector.scalar_tensor_tensor(
                            out_ap, h_in, lnb_sb[:, t : t + 1], b_in,
                            op0=OP.add, op1=OP.subtract,
                        )

            # PSUM budget (8 banks): mean 1 + msq 1 (reused by l) +
            # m0 1 + m1 1 (conv, reused by po) + pvt 1 + ps 3 = 8.
            with (
                tc.tile_pool(name="xio", bufs=2) as xio,
                tc.tile_pool(name="ltmp", bufs=2) as ltmp,
                tc.tile_pool(name="lbc", bufs=1) as lbc,
                tc.tile_pool(name="cwp", bufs=1) as cwp,
                tc.tile_pool(name="vsl", bufs=2) as vsl,
                tc.tile_pool(name="app", bufs=40) as app,
                tc.tile_pool(name="zout", bufs=2) as zout,
                tc.tile_pool(name="lout", bufs=2) as lout,
                tc.tile_pool(name="lps", bufs=1, space="PSUM") as lps,
                tc.tile_pool(name="kps", bufs=1, space="PSUM") as kps,
                tc.tile_pool(name="aps", bufs=3, space="PSUM") as aps,
            ):
                P = [xio, ltmp, lps, lbc]

                # ---- strip LN chunks -> padded hkv tile
                strip_chunks = []
                done = 0
                while done < KVR * W:
                    KC = min(MC, KVR * W - done)
                    strip_chunks.append((done, KC))
                    done += KC

                def emit_strip_chunk(c, nsplit=2):
                    off, KC = c
                    r0, nr = off // W, KC // W
                    ln_chunk(
                        xkv_d[:], slice(off, off + KC), KC, P,
                        lambda t: hkv_sb[:, t, r0 : r0 + nr, 1 : W + 1],
                        out_rearrange=("p (r w) -> p r w", dict(w=W)),
                        nsplit=nsplit,
                    )

                # LN chunk for non-owned queries -> qh_sb
                def emit_q_chunk(lc):
                    lsl = slice(lc * MC, (lc + 1) * MC)
                    ln_chunk(xq_d[:], lsl, MC, P, lambda t: qh_sb[:, t, lsl])

                # ---- direct 3x3 conv, one slab = RS rows x W cols = MC pix.
                # moving = 4D padded window; 18 DoubleRow accumulation steps.
                def emit_conv_slab(w_sb, s, dest_fn):
                    for ot in range(CT):
                        pk = kps.tile((128, MC), F32, tag=f"m{ot % 2}",
                                      name="pk")
                        i = 0
                        for dy in range(3):
                            for dx in range(3):
                                win = hkv_sb[:, :, RS * s + dy : RS * s + dy + RS,
                                             dx : dx + W]
                                for tp in range(CT // 2):
                                    nc.tensor.matmul(
                                        pk,
                                        w_sb[:, (dy * 3 + dx) * CT + 2 * tp :
                                             (dy * 3 + dx) * CT + 2 * tp + 2,
                                             ot * 128 : ot * 128 + 128],
                                        win[:, 2 * tp : 2 * tp + 2],
                                        start=(i == 0), stop=(i == 17),
                                        perf_mode=DR,
                                    )
                                    i += 1
                        dest_fn(ot, pk)

                def k_dest(s):
                    ksl = slice(s * MC, (s + 1) * MC)
                    def dest(ot, pk):
                        nc.vector.tensor_copy(k_sb[:, ot, ksl], pk)
                        # fp8 residual: writing the sub result to fp8 IS the
                        # second-level quantization
                        nc.vector.tensor_sub(k_lo[:, ot, ksl], pk,
                                             k_sb[:, ot, ksl])
                    return dest

                def emit_v_slab(s):
                    vslab = vsl.tile((128, CT, MC), BF16, tag="vslab",
                                     name="vslab")

                    def dest(ot, pk):
                        nc.scalar.copy(vslab[:, ot], pk)

                    emit_conv_slab(wv_sb, s, dest)
                    for blk in range(MC // 128):
                        n_idx = s * (MC // 128) + blk
                        pvt = kps.tile((128, C), BF16, tag="pvt",
                                       name="pvt", bufs=1)
                        for ct in range(CT):
                            nc.tensor.transpose(
                                pvt[:, ct * 128 : ct * 128 + 128],
                                vslab[:, ct, blk * 128 : (blk + 1) * 128], ident)
                        nc.scalar.copy(vT_sb[:, n_idx], pvt)
                        nc.vector.tensor_sub(vT_lo[:, n_idx], pvt,
                                             vT_sb[:, n_idx])

                # ---- attention chunk pieces (LOCAL chunk indexing: chunks
                # 0..3 are this core's own key-half rows (read from the strip
                # tile), 4..7 the other half (read from qh_sb).  The host
                # permutes z/l back to global order per core half.
                def q_src(i, tp):
                    if i < NCH // 2:
                        r = RS * i + 1
                        return hkv_sb[:, 2 * tp : 2 * tp + 2, r : r + RS,
                                      1 : W + 1]
                    lc = i - NCH // 2
                    return qh_sb[:, 2 * tp : 2 * tp + 2,
                                 lc * MC : (lc + 1) * MC]

                p_stash = {}

                def emit_scores(i):
                    ps_l = []
                    for j in range(NT // 2):
                        p2 = app.tile((128, 2, MC), F8, tag="p", name="p2")
                        ps2 = aps.tile((128, 2, MC), F32, tag="ps", name="ps2",
                                       bufs=2)
                        for h2 in range(2):
                            n = 2 * j + h2
                            st = 0
                            for klv in (k_sb, k_lo):
                                for tp in range(CT // 2):
                                    nc.tensor.matmul(
                                        ps2[:, h2], klv[:, 2 * tp : 2 * tp + 2,
                                                        n * 128 : (n + 1) * 128],
                                        q_src(i, tp),
                                        start=(st == 0), stop=(st == CT - 1),
                                        perf_mode=DR,
                                    )
                                    st += 1
                        # one exp covers both halves of the pair
                        nc.scalar.activation(
                            p2.rearrange("p a n -> p (a n)"),
                            ps2.rearrange("p a n -> p (a n)"),
                            AF.Exp, bias=expb_t, scale=1.0 / WKS)
                        ps_l.append(p2)
                    p_stash[i] = ps_l

                def emit_pv(i):
                    msl = slice(i * MC, (i + 1) * MC)
                    ps_l = p_stash.pop(i)
                    for ct in range(CT):
                        po = kps.tile((128, MC), F32, tag=f"m{ct % 2}",
                                      name="po", bufs=1)
                        st = 0
                        for j in range(NT // 2):
                            for vlv in (vT_sb, vT_lo):
                                nc.tensor.matmul(
                                    po, vlv[:, 2 * j : 2 * j + 2,
                                            ct * 128 : ct * 128 + 128],
                                    ps_l[j], start=(st == 0), stop=(st == NT - 1),
                                    perf_mode=DR,
                                )
                                st += 1
                        z_sb = zout.tile((128, MC), BF16, tag=f"z{ct % 2}",
                                         name="z_sb")
                        nc.vector.tensor_copy(z_sb, po)  # 2^-5 folded on host
                        nc.sync.dma_start(
                            out=z_d[ct * 128 : ct * 128 + 128, msl], in_=z_sb)
                    # every output row of this DoubleRow matmul is sum_n p;
                    # reuses the m0 conv/PV psum bank, row 0 is copied out.
                    l_ps = kps.tile((128, MC), F32, tag="m0", name="l_ps",
                                    bufs=1)
                    for j in range(NT // 2):
                        nc.tensor.matmul(l_ps, ones8, ps_l[j],
                                         start=(j == 0), stop=(j == NT // 2 - 1),
                                         perf_mode=DR)
                    l_sb = lout.tile((1, MC), F32, tag="lsb", name="l_sb")
                    nc.scalar.copy(l_sb, l_ps[0:1])
                    nc.sync.dma_start(out=l_d[:, msl], in_=l_sb)

                # ---- emission schedule ------------------------------------
                nc.sync.dma_start(
                    out=lnb_sb, in_=lnb_d[:].rearrange("(t p) o -> p (t o)", p=128)
                )
                wk_sb = cwp.tile((128, 9 * CT, C), F8, tag="cw", name="wk_sb")
                emit_strip_chunk(strip_chunks[0], nsplit=4)
                nc.sync.dma_start(out=wk_sb[:, : 3 * CT], in_=wk_d[:, : 3 * CT])
                emit_strip_chunk(strip_chunks[1], nsplit=4)
                nc.sync.dma_start(out=wk_sb[:, 3 * CT : 6 * CT],
                                  in_=wk_d[:, 3 * CT : 6 * CT])
                emit_strip_chunk(strip_chunks[2])
                nc.sync.dma_start(out=wk_sb[:, 6 * CT :], in_=wk_d[:, 6 * CT :])
                emit_strip_chunk(strip_chunks[3])
                emit_strip_chunk(strip_chunks[4])

                for s in range(NSLAB):
                    emit_conv_slab(wk_sb, s, k_dest(s))
                    emit_q_chunk(s)

                wv_sb = cwp.tile((128, 9 * CT, C), F8, tag="cw2", name="wv_sb")
                for j in range(3):
                    nc.sync.dma_start(out=wv_sb[:, 3 * j * CT : 3 * (j + 1) * CT],
                                      in_=wv_d[:, 3 * j * CT : 3 * (j + 1) * CT])

                # scores for owned chunks overlap the v conv; PV waits on vT.
                for s in range(NSLAB):
                    emit_scores(s)
                    emit_v_slab(s)
                emit_pv(0)
                for s in range(NSLAB):
                    emit_scores(NCH // 2 + s)
                    emit_pv(s + 1)
                emit_pv(NCH // 2 + 1)
                emit_pv(NCH // 2 + 2)
                emit_pv(NCH // 2 + 3)

    nc.compile()
    return nc


_NC_CACHE = {}


def _get_nc(C, H, W, lnb_zero=False):
    key = (C, H, W, lnb_zero)
    if key not in _NC_CACHE:
        _NC_CACHE[key] = build_attn_kernel(C, H, W, lnb_zero=lnb_zero)
    return _NC_CACHE[key]


def make_in_maps(x, ln_w, ln_b, wq, wk, wv, wp, bp, n_cores=8):
    """Host-side prep: shard + relayout inputs for each core."""
    x = np.asarray(x, np.float32)
    B, C, H, W_ = x.shape
    HW = H * W_
    KH = H // 2
    CT = C // 128
    scale = float(C) ** -0.5
    lnw = np.asarray(ln_w, np.float32).reshape(C)
    F8NP = ml_dtypes.float8_e4m3

    # composite conv weights: k2 = (wq*scale . wk) (*) h, v2 = (wp . wv) (*) h
    wq2 = np.asarray(wq, np.float32)[:, :, 0, 0] * scale       # [O, C]
    wpm = np.asarray(wp, np.float32)[:, :, 0, 0]               # [O, C]
    wk9 = np.asarray(wk, np.float32).reshape(C, C, 9)          # [O, I, tap]
    wv9 = np.asarray(wv, np.float32).reshape(C, C, 9)

    def _composite(m1, w9, s, transpose_m1):
        # w2[o, i, tap] = sum_c m1[c|o, o|c] * w9[c, i, tap] * lnw[i] * s
        ein = "co,cit->oit" if transpose_m1 else "oc,cit->oit"
        w2 = np.einsum(ein, m1, w9) * lnw[None, :, None] * s
        # layout [p, tap*CT + t, o] with i = t*128 + p
        arr = w2.transpose(1, 2, 0).reshape(CT, 128, 9, C)     # [t, p, tap, o]
        arr = arr.transpose(1, 2, 0, 3).reshape(128, 9 * CT, C)
        return np.ascontiguousarray(arr.astype(F8NP))

    # logits = h_q^T (Wq^T k), so the q fold uses Wq TRANSPOSED; the proj
    # fold (out = Wp attn_out) uses Wp as-is.
    wkT = _composite(wq2, wk9, WKS, transpose_m1=True)
    wvT = _composite(wpm, wv9, WVS, transpose_m1=False)
    lnb = np.ascontiguousarray(np.asarray(ln_b, np.float32).reshape(C, 1))
    xi = x.reshape(B, C, H, W_)
    in_maps = []
    for core in range(n_cores):
        b, half = divmod(core, 2)
        b = b % B
        zero = np.zeros((C, 1, W_), np.float32)
        if half == 0:
            strip = np.concatenate([zero, xi[b][:, 0 : KH + 1]], axis=1)
            xq = xi[b][:, KH:H]
        else:
            strip = np.concatenate([xi[b][:, KH - 1 : H], zero], axis=1)
            xq = xi[b][:, 0:KH]
        in_maps.append({
            "xq": np.ascontiguousarray(
                xq.reshape(C, HW // 2).astype(ml_dtypes.bfloat16)),
            "xkv": np.ascontiguousarray(
                strip.reshape(C, (KH + 2) * W_).astype(ml_dtypes.bfloat16)),
            "wk": wkT, "wv": wvT, "lnb": lnb,
        })
    return in_maps


def merge_outputs(x, bp, results):
    """Exact pair-merge: y = x + (Z_a + Z_b) / (l_a + l_b) + bp.

    Cores write queries in LOCAL order (own key-half rows first); half-1
    cores therefore need their z/l swapped back to global row order."""
    x = np.asarray(x, np.float32)
    B, C, H, W_ = x.shape
    HW = H * W_
    hh = HW // 2
    bp = np.asarray(bp, np.float32).reshape(C, 1)

    def _glob(res, half):
        z = res["z"].astype(np.float32)
        l = np.asarray(res["l"], np.float32)
        if half == 1:
            z = np.concatenate([z[:, hh:], z[:, :hh]], axis=1)
            l = np.concatenate([l[:, hh:], l[:, :hh]], axis=1)
        return z, l

    out = np.empty((B, C, HW), np.float32)
    for b in range(B):
        za, la = _glob(results[2 * b], 0)
        zb, lb = _glob(results[2 * b + 1], 1)
        out[b] = x.reshape(B, C, HW)[b] + (za + zb) / (WVS * (la + lb)) + bp
    return out.reshape(B, C, H, W_)


def kernel(x, ln_w, ln_b, wq, wk, wv, wp, bp):
    from concourse.bass_utils import run_bass_kernel_spmd

    x = np.asarray(x, np.float32)
    B, C, H, W_ = x.shape
    lnb_zero = bool((np.asarray(ln_b, np.float32) == 0).all())
    nc = _get_nc(C, H, W_, lnb_zero=lnb_zero)
    in_maps = make_in_maps(x, ln_w, ln_b, wq, wk, wv, wp, bp)
    res = run_bass_kernel_spmd(nc, in_maps, core_ids=list(range(8)))
    return merge_outputs(x, bp, res.results)
